# revision 80
# baseline (speedup 1.0000x reference)
"""Trainium2 Bass kernel for nn_AdaptiveSequenceProcessor.

Math (see reference):
  context  = mean_s(features)                               [B, D]
  span_w   = softmax(MLP_sp(context))                       [B, 3]
  feat_l   = relevance_pool(features[-32:],  context, 0.9)  [B, D]
  feat_g   = relevance_pool(features[::128], context, 0.8)  [B, D]
  fused    = LN(gelu(W_ff @ [feat_l*sw1 | feat_g*sw2]))     [B, D]
  gate     = sigmoid(W_g @ fused)  (constant over s!)       [B, D]
  out      = LN(features + fused*gate)                      [S, B, D]

Key structural fact: fused_expanded/gate are constant along the sequence
axis, so per batch element the heavy work is one mean over S (pass 1)
and one LN(features + c_b) sweep (pass 2); everything else is tiny.

Sharding: data-parallel over batch. 16 batch elements / 8 cores = 2 per
core, weights replicated, no collectives needed.

With FEAT_BF16: features are pre-cast to bf16 on host (halves the HBM
read), pass-1 tiles stay resident in SBUF and pass 2 reads them from
SBUF (no second HBM read). w_fft is streamed from DRAM per use to make
room for the cache.
"""

import os
import sys

sys.path.insert(0, "/opt/trn_rl_repo")

import numpy as np
import ml_dtypes

import concourse.bass as bass
import concourse.tile as tile
from concourse import bacc, mybir
from concourse.bass_utils import run_bass_kernel_spmd

F32 = mybir.dt.float32
BF16 = mybir.dt.bfloat16
AF = mybir.ActivationFunctionType
ALU = mybir.AluOpType
AX = mybir.AxisListType

S, B, D, H = 4096, 16, 1024, 512
P = 128          # SBUF partitions
NC = 8           # cores
BPC = B // NC    # batch per core = 2
NT = S // P      # 32 sequence tiles of 128
L = 32           # pool history length
DC = D // P      # 8 d-chunks of 128
HC = H // P      # 4 h-chunks of 128
LN_EPS = 1e-5
CACHE_SLOTS = 16   # pass-1 tile-pairs of b0 kept resident for pass 2
NTP = NT // 2    # pass-1/2 work in pairs of 128-row tiles: [128, 2048]

# Feature flags (module-level so test.py can flip them for experiments)
FLAGS = dict(
    feat_bf16=True,    # bf16 features input + SBUF cache + streamed w_fft
    use_ttr=False,     # tensor_tensor_reduce — CRASHES HW (exec unit
                       # unrecoverable), never enable
    use_sq_accum=True,  # ACT Square with accum_out + Identity bias/scale apply
    use_pe_transpose=False,  # is_transpose matmuls instead of identity matmul
    out_bf16=True,     # write output as bf16, upcast on host (halves write BW)
)


def _build(apply_ln_affine: bool, feat_bf16: bool, use_ttr: bool,
           use_sq_accum: bool, use_pe_transpose: bool, out_bf16: bool = True,
           reps: int = 1):
    """Build the per-core Bass graph. Returns compiled Bacc.

    reps > 1 wraps the whole kernel in a tc.For_i hardware loop — used
    only by the timing harness to amortize per-call dispatch overhead.
    """
    nc = bacc.Bacc("TRN2", target_bir_lowering=False, debug=False,
                   num_devices=NC)

    FDT = BF16 if feat_bf16 else F32

    feat = nc.declare_dram_parameter("features", [S, BPC, D], FDT, isOutput=False)
    w_sp1t = nc.declare_dram_parameter("w_sp1t", [D, H], BF16, isOutput=False)
    b_sp1t = nc.declare_dram_parameter("b_sp1t", [P, HC], F32, isOutput=False)
    w_sp2t = nc.declare_dram_parameter("w_sp2t", [H, 3], BF16, isOutput=False)
    b_sp2 = nc.declare_dram_parameter("b_sp2", [1, 3], F32, isOutput=False)
    w_rs1t = nc.declare_dram_parameter("w_rs1t", [2 * D, H], BF16, isOutput=False)
    b_rs1 = nc.declare_dram_parameter("b_rs1", [1, H], F32, isOutput=False)
    w_rs2r = nc.declare_dram_parameter("w_rs2r", [1, H], F32, isOutput=False)
    dlb = nc.declare_dram_parameter("dlb", [1, L], F32, isOutput=False)
    dgb = nc.declare_dram_parameter("dgb", [1, L], F32, isOutput=False)
    w_fft = nc.declare_dram_parameter("w_fft", [2 * D, D], BF16, isOutput=False)
    b_ff = nc.declare_dram_parameter("b_ff", [1, D], F32, isOutput=False)
    lnffg = nc.declare_dram_parameter("lnffg", [1, D], F32, isOutput=False)
    lnffb = nc.declare_dram_parameter("lnffb", [1, D], F32, isOutput=False)
    w_gatet = nc.declare_dram_parameter("w_gatet", [D, D], BF16, isOutput=False)
    b_gate = nc.declare_dram_parameter("b_gate", [1, D], F32, isOutput=False)
    i32d = nc.declare_dram_parameter("i32", [L, L], F32, isOutput=False)
    if feat_bf16:
        i32bd = nc.declare_dram_parameter("i32b", [L, L], BF16, isOutput=False)
    if apply_ln_affine:
        lng = nc.declare_dram_parameter("lng", [1, D], F32, isOutput=False)
        lnb = nc.declare_dram_parameter("lnb", [1, D], F32, isOutput=False)
    ODT = BF16 if out_bf16 else F32
    out = nc.declare_dram_parameter("out", [S, BPC, D], ODT, isOutput=True)

    w_fft_re = w_fft.ap().rearrange("(c p) n -> p c n", p=P)
    w_gatet_re = w_gatet.ap().rearrange("(c p) n -> p c n", p=P)

    with tile.TileContext(nc) as tc:
        from contextlib import ExitStack
        with ExitStack() as ctx:
            consts = ctx.enter_context(tc.tile_pool(name="consts", bufs=1))
            wpool = ctx.enter_context(tc.tile_pool(name="weights", bufs=1))
            bpool = ctx.enter_context(tc.tile_pool(name="perb", bufs=1))
            scp = ctx.enter_context(tc.tile_pool(name="sc", bufs=1))
            rows = ctx.enter_context(tc.tile_pool(name="rows", bufs=1))
            if feat_bf16:
                cachep = ctx.enter_context(tc.tile_pool(name="cache", bufs=CACHE_SLOTS))
                wffp = ctx.enter_context(tc.tile_pool(name="wff", bufs=3))
                wgp = ctx.enter_context(tc.tile_pool(name="wg", bufs=3))
                p1sp = ctx.enter_context(tc.tile_pool(name="p1s", bufs=3))
            else:
                p1p = ctx.enter_context(tc.tile_pool(name="p1", bufs=4))
            p2p = ctx.enter_context(tc.tile_pool(name="p2", bufs=2))
            stp = ctx.enter_context(tc.tile_pool(name="stats", bufs=3))
            psS = ctx.enter_context(tc.tile_pool(name="psS", bufs=5, space="PSUM"))
            psC = ctx.enter_context(tc.tile_pool(name="psC", bufs=1, space="PSUM"))

            # ---- constants ----
            ones128 = consts.tile([P, 1], FDT)
            nc.vector.memset(ones128[:], 1.0)
            one11 = consts.tile([1, 1], F32)
            nc.vector.memset(one11[:], 1.0)
            ones_1x32_bf = consts.tile([1, L], BF16)
            nc.vector.memset(ones_1x32_bf[:], 1.0)
            ones_1x128 = consts.tile([1, P], F32)
            nc.vector.memset(ones_1x128[:], 1.0)
            i32sb = consts.tile([L, L], F32)
            nc.scalar.dma_start(i32sb[:], i32d[:])
            if feat_bf16:
                i32bsb = consts.tile([L, L], BF16)
                nc.scalar.dma_start(i32bsb[:], i32bd[:])
            eps_sb = consts.tile([P, 1], F32)
            nc.vector.memset(eps_sb[:], LN_EPS)

            # ---- weights to SBUF (gpsimd/SWDGE queue, ordered by need;
            # the sync queue stays free for feature loads) ----
            w_sp1t_sb = wpool.tile([P, DC, H], BF16)
            nc.scalar.dma_start(w_sp1t_sb[:], w_sp1t.ap().rearrange("(c p) n -> p c n", p=P))
            w_rs1t_sb = wpool.tile([P, 2 * DC, H], BF16)
            nc.scalar.dma_start(w_rs1t_sb[:], w_rs1t.ap().rearrange("(c p) n -> p c n", p=P))
            b_sp1t_sb = wpool.tile([P, HC], F32)
            nc.scalar.dma_start(b_sp1t_sb[:], b_sp1t[:])
            w_sp2t_sb = wpool.tile([P, HC, 3], BF16)
            nc.scalar.dma_start(w_sp2t_sb[:], w_sp2t.ap().rearrange("(c p) n -> p c n", p=P))
            if not feat_bf16:
                w_fft_sb = wpool.tile([P, 2 * DC, D], BF16)
                nc.scalar.dma_start(w_fft_sb[:], w_fft_re)
                w_gatet_sb = wpool.tile([P, DC, D], BF16)
                nc.scalar.dma_start(w_gatet_sb[:], w_gatet.ap().rearrange("(c p) n -> p c n", p=P))

            b_sp2_sb = consts.tile([1, 3], F32)
            nc.scalar.dma_start(b_sp2_sb[:], b_sp2[:])
            b_rs1_sb = consts.tile([1, H], F32)
            nc.scalar.dma_start(b_rs1_sb[:], b_rs1[:])
            w_rs2r_sb = consts.tile([1, H], F32)
            nc.scalar.dma_start(w_rs2r_sb[:], w_rs2r[:])
            dlb_sb = consts.tile([1, L], F32)
            nc.scalar.dma_start(dlb_sb[:], dlb[:])
            dgb_sb = consts.tile([1, L], F32)
            nc.scalar.dma_start(dgb_sb[:], dgb[:])
            # four [1,D] const rows packed at quadrant partitions of one
            # tile (engine operands accept start partitions 0/32/64/96)
            constrows = consts.tile([P, 2, D], F32)
            nc.scalar.dma_start(constrows[0:1, 0, :], b_ff[:])
            nc.scalar.dma_start(constrows[0:1, 1, :], lnffb[:])
            nc.scalar.dma_start(constrows[32:33, 0, :], lnffg[:])
            nc.scalar.dma_start(constrows[64:65, 0, :], b_gate[:])
            b_ff_sb = constrows[0:1, 0, :]
            lnffb_sb = constrows[0:1, 1, :]
            lnffg_sb = constrows[32:33, 0, :]
            b_gate_sb = constrows[64:65, 0, :]

            # w_rs2 broadcast to 32 partitions (one-time)
            pw32 = psS.tile([L, H], F32, tag="ps")
            ones_1x32_f = consts.tile([1, L], F32)
            nc.vector.memset(ones_1x32_f[:], 1.0)
            nc.tensor.matmul(pw32[:], lhsT=ones_1x32_f[:], rhs=w_rs2r_sb[:],
                             start=True, stop=True)
            w_rs2b = consts.tile([L, H], F32)
            nc.scalar.copy(w_rs2b[:], pw32[:])
            # w_rs2 as [128, HC] bf16 columns (for the transposed-h scores)
            pwc4 = psS.tile([P, HC], F32, tag="ps")
            for jh in range(HC):
                nc.tensor.matmul(pwc4[:, jh:jh + 1],
                                 lhsT=w_rs2r_sb[:, jh * P:(jh + 1) * P],
                                 rhs=one11[:], start=True, stop=True)
            w_rs2c_sb = consts.tile([P, HC], BF16)
            nc.scalar.copy(w_rs2c_sb[:], pwc4[:])

            if apply_ln_affine:
                lng_sb = consts.tile([1, D], F32)
                nc.scalar.dma_start(lng_sb[:], lng[:])
                lnb_sb = consts.tile([1, D], F32)
                nc.scalar.dma_start(lnb_sb[:], lnb[:])
                g_bcast2 = bpool.tile([P, 2, D], F32)
                b_bcast2 = bpool.tile([P, 2, D], F32)
                for h2 in range(2):
                    pgb = psS.tile([P, 512], F32, tag="ps")
                    nc.tensor.matmul(pgb[:], lhsT=ones_1x128[:],
                                     rhs=lng_sb[:, h2 * 512:(h2 + 1) * 512],
                                     start=True, stop=True)
                    pbb = psS.tile([P, 512], F32, tag="ps")
                    nc.tensor.matmul(pbb[:], lhsT=ones_1x128[:],
                                     rhs=lnb_sb[:, h2 * 512:(h2 + 1) * 512],
                                     start=True, stop=True)
                    for hh in range(2):
                        nc.scalar.copy(g_bcast2[:, hh, h2 * 512:(h2 + 1) * 512],
                                       pgb[:])
                        nc.scalar.copy(b_bcast2[:, hh, h2 * 512:(h2 + 1) * 512],
                                       pbb[:])

            CBD = BF16 if feat_bf16 else F32
            c_bcast = [bpool.tile([P, 2, D], CBD, tag=f"cb{b}",
                       name=f"c_bcast{b}") for b in range(BPC)]
            cb2 = c_bcast
            ctxT_bf = [bpool.tile([P, DC], BF16, tag=f"ctxT{b}", name=f"ctxT{b}")
                       for b in range(BPC)]

            def pe_transpose(out_psum, in_sb, ident):
                if use_pe_transpose:
                    nc.tensor.transpose(out_psum, in_sb, ident)
                else:
                    nc.tensor.matmul(out_psum, lhsT=in_sb, rhs=ident,
                                     start=True, stop=True)

            # identity for transposing FDT-typed tiles
            i32f = i32bsb if feat_bf16 else i32sb

            # ================= PASS 1: context mean =================
            xtiles = [[None] * NT for _ in range(BPC)]

            def pass1_loads(b):
                # b=0 tiles land in the resident cache (read again by pass
                # 2); b=1 tiles stream through a small pool (pass 2
                # re-reads from HBM) so b1's pass 1 is decoupled from b0's
                # pass 2 progress.
                for i in range(NTP):
                    if feat_bf16:
                        if b == 0:
                            x = cachep.tile([P, 2, D], FDT, tag="xc",
                                            name=f"x_{b}_{i}")
                        else:
                            x = p1sp.tile([P, 2, D], FDT, tag="xs",
                                          name=f"x_{b}_{i}")
                    else:
                        x = p1p.tile([P, 2, D], FDT, tag="x", name=f"x_{b}_{i}")
                    xtiles[b][i] = x
                    src = feat[i * 2 * P:(i + 1) * 2 * P, b, :].rearrange(
                        "(h p) d -> p h d", p=P)
                    nc.sync.dma_start(x[:], src)

            def pass1_mms(b):
                pc = psC.tile([1, D], F32, tag="pc")  # 2 banks
                for i in range(NTP):
                    x = xtiles[b][i]
                    for hh in range(2):
                        for h2 in range(2):
                            nc.tensor.matmul(
                                pc[:, h2 * 512:(h2 + 1) * 512],
                                lhsT=ones128[:],
                                rhs=x[:, hh, h2 * 512:(h2 + 1) * 512],
                                start=(i == 0 and hh == 0),
                                stop=(i == NTP - 1 and hh == 1))
                ctx_row = scp.tile([1, D], F32, tag="ctxrow")
                nc.scalar.mul(ctx_row[:], pc[:], 1.0 / S)
                pct = psS.tile([P, DC], F32, tag="ps")
                for j in range(DC):
                    pe_transpose(pct[:, j:j + 1],
                                 ctx_row[:, j * P:(j + 1) * P], one11[:])
                nc.scalar.copy(ctxT_bf[b][:], pct[:])

            # ================= SMALL COMPUTE per b =================
            def sc_part1(b):
                # --- phase A: ctx-independent hist loads + transposes ---
                hists, histTs = [], []
                for pi in range(2):
                    hist = scp.tile([L, D], FDT, tag=f"hist{pi}",
                                    name=f"hist{pi}")
                    if pi == 0:
                        nc.sync.dma_start(hist[:], feat[S - L:S, b, :])
                    else:
                        nc.sync.dma_start(hist[:], feat[0:S:P, b, :])
                    hists.append(hist)
                for pi in range(2):
                    histT_bf = scp.tile([P, DC, L], BF16, tag=f"histT{pi}",
                                        name=f"histT{pi}")
                    for jd in range(DC):
                        pt = psS.tile([P, L], F32, tag="ps")
                        pe_transpose(pt[:], hists[pi][:, jd * P:(jd + 1) * P],
                                     i32f[:])
                        nc.scalar.copy(histT_bf[:, jd, :], pt[:])
                    histTs.append(histT_bf)
                    if not feat_bf16:
                        hist_bf = scp.tile([L, D], BF16, tag=f"histbf{pi}",
                                           name=f"histbf{pi}")
                        nc.scalar.copy(hist_bf[:], hists[pi][:])
                        hists[pi] = hist_bf

                # --- span MLP (needs ctx) ---
                psp = psS.tile([P, HC], F32, tag="ps")
                for jh in range(HC):
                    for jd in range(DC):
                        nc.tensor.matmul(
                            psp[:, jh:jh + 1],
                            lhsT=w_sp1t_sb[:, jd, jh * P:(jh + 1) * P],
                            rhs=ctxT_bf[b][:, jd:jd + 1],
                            start=(jd == 0), stop=(jd == DC - 1))
                sp_hT = scp.tile([P, HC], BF16, tag="sphT")
                for jh in range(HC):
                    nc.scalar.activation(sp_hT[:, jh:jh + 1], psp[:, jh:jh + 1],
                                         AF.Gelu, bias=b_sp1t_sb[:, jh:jh + 1])
                psl = psS.tile([1, 3], F32, tag="ps")
                for jh in range(HC):
                    nc.tensor.matmul(psl[:], lhsT=sp_hT[:, jh:jh + 1],
                                     rhs=w_sp2t_sb[:, jh, :],
                                     start=(jh == 0), stop=(jh == HC - 1))
                # softmax over 3 logits (values are O(1): no max-subtract)
                e3 = rows.tile([1, 3], F32, tag="e3")
                nc.vector.tensor_add(e3[:], psl[:], b_sp2_sb[:])
                nc.scalar.activation(e3[:], e3[:], AF.Exp)
                z3 = stp.tile([1, 1], F32, tag="z3")
                nc.vector.reduce_sum(z3[:], e3[:], AX.X)
                rz3 = stp.tile([1, 1], F32, tag="rz3")
                nc.vector.reciprocal(rz3[:], z3[:])
                sw = rows.tile([1, 3], F32, tag="sw")
                nc.vector.tensor_scalar(sw[:], e3[:], rz3[:], None, op0=ALU.mult)

                # --- ctx part of relevance-MLP (shared by both pools) ---
                pcr = psS.tile([1, H], F32, tag="ps")
                for jd in range(DC):
                    nc.tensor.matmul(pcr[:], lhsT=ctxT_bf[b][:, jd:jd + 1],
                                     rhs=w_rs1t_sb[:, DC + jd, :],
                                     start=(jd == 0), stop=(jd == DC - 1))
                ctx_rs_bf = scp.tile([1, H], BF16, tag="ctxrs")
                nc.vector.tensor_add(ctx_rs_bf[:], pcr[:], b_rs1_sb[:])

                # --- two relevance pools, phase-interleaved ---
                srows, wcols = [], []
                # hidden layer computed TRANSPOSED (h^T[hh, l]) so the
                # score reduction over hh happens on PE (partition
                # contraction) instead of a DVE mul+reduce + transpose.
                hTs = []
                for pi in range(2):
                    phT = psS.tile([P, HC, L], F32, tag="ps",
                                   name=f"phT{pi}")
                    for jh in range(HC):
                        for jd in range(DC):
                            nc.tensor.matmul(
                                phT[:, jh, :],
                                lhsT=w_rs1t_sb[:, jd, jh * P:(jh + 1) * P],
                                rhs=histTs[pi][:, jd, :],
                                start=(jd == 0), stop=False)
                        nc.tensor.matmul(
                            phT[:, jh, :],
                            lhsT=ctx_rs_bf[:, jh * P:(jh + 1) * P],
                            rhs=ones_1x32_bf[:], start=False, stop=True)
                    hT = scp.tile([P, HC, L], BF16, tag=f"hT{pi}",
                                  name=f"hT{pi}")
                    nc.scalar.activation(hT[:], phT[:], AF.Gelu)
                    hTs.append(hT)
                for pi, dbias in enumerate([dlb_sb, dgb_sb]):
                    psc = psS.tile([1, L], F32, tag="ps", name=f"psc{pi}")
                    for jh in range(HC):
                        nc.tensor.matmul(psc[:], lhsT=w_rs2c_sb[:, jh:jh + 1],
                                         rhs=hTs[pi][:, jh, :],
                                         start=(jh == 0), stop=(jh == HC - 1))
                    # scores are O(1)-ish and the decay bias is <= 0: exp is
                    # safe without max-subtraction.
                    e32 = rows.tile([1, L], F32, tag=f"e32{pi}",
                                    name=f"e32{pi}")
                    nc.vector.tensor_add(e32[:], psc[:], dbias[:])
                    nc.scalar.activation(e32[:], e32[:], AF.Exp)
                    z32 = stp.tile([1, 1], F32, tag=f"z32{pi}",
                                   name=f"z32{pi}")
                    nc.vector.reduce_sum(z32[:], e32[:], AX.X)
                    rz32 = stp.tile([1, 1], F32, tag=f"rz32{pi}",
                                    name=f"rz32{pi}")
                    nc.vector.reciprocal(rz32[:], z32[:])
                    wrow = rows.tile([1, L], F32, tag=f"wrow{pi}",
                                     name=f"wrow{pi}")
                    nc.vector.tensor_scalar(wrow[:], e32[:], rz32[:], None,
                                            op0=ALU.mult)
                    srows.append(wrow)
                featT = []
                for pi in range(2):
                    pwc = psS.tile([L, 1], F32, tag="ps", name=f"pwc{pi}")
                    pe_transpose(pwc[:], srows[pi][:], one11[:])
                    wcol_bf = stp.tile([L, 1], BF16, tag=f"wcol{pi}",
                                       name=f"wcol{pi}")
                    nc.scalar.copy(wcol_bf[:], pwc[:])
                    wcols.append(wcol_bf)
                for pi in range(2):
                    pft = psS.tile([P, DC], F32, tag="ps", name=f"pft{pi}")
                    for jd in range(DC):
                        nc.tensor.matmul(pft[:, jd:jd + 1],
                                         lhsT=hists[pi][:, jd * P:(jd + 1) * P],
                                         rhs=wcols[pi][:], start=True, stop=True)
                    fT = scp.tile([P, DC], BF16, tag=f"featT{pi}",
                                  name=f"featT{pi}")
                    nc.scalar.copy(fT[:], pft[:])
                    featT.append(fT)
                return featT, sw

            def sc_part2(b, featT, sw):
                # --- fusion ff ---
                pAB = []
                for pi in range(2):
                    pAh = [psS.tile([1, 512], F32, tag="ps", name=f"pA{pi}{h2}")
                           for h2 in range(2)]
                    for jd in range(DC):
                        if feat_bf16:
                            wff_c = wffp.tile([P, D], BF16, tag="wff",
                                              name=f"wffc{pi}{jd}")
                            nc.gpsimd.dma_start(wff_c[:],
                                                w_fft_re[:, pi * DC + jd, :])
                        else:
                            wff_c = w_fft_sb[:, pi * DC + jd, :]
                        for h2 in range(2):
                            nc.tensor.matmul(
                                pAh[h2][:], lhsT=featT[pi][:, jd:jd + 1],
                                rhs=wff_c[:, h2 * 512:(h2 + 1) * 512],
                                start=(jd == 0), stop=(jd == DC - 1))
                    pAB.append(pAh)
                rowq = scp.tile([P, D], F32, tag="rowq")
                t1 = rowq[0:1, :]
                for h2 in range(2):
                    nc.vector.tensor_scalar(t1[:, h2 * 512:(h2 + 1) * 512],
                                            pAB[0][h2][:], sw[:, 1:2], None,
                                            op0=ALU.mult)
                    # scale the global half in PSUM, then accumulate from
                    # PSUM (PSUM operand is exempt from the equal-base rule)
                    nc.vector.tensor_scalar(pAB[1][h2][:], pAB[1][h2][:],
                                            sw[:, 2:3], None, op0=ALU.mult)
                    nc.vector.tensor_add(t1[:, h2 * 512:(h2 + 1) * 512],
                                         pAB[1][h2][:],
                                         t1[:, h2 * 512:(h2 + 1) * 512])
                nc.vector.tensor_add(t1[:], t1[:], b_ff_sb[:])
                ff = rowq[64:65, :]  # q2
                nc.scalar.activation(ff[:], t1[:], AF.Gelu)
                # LN over free axis
                smu = stp.tile([1, 1], F32, tag="smu")
                nc.vector.reduce_sum(smu[:], ff[:], AX.X)
                mu = stp.tile([1, 1], F32, tag="mu")
                nc.vector.tensor_scalar(mu[:], smu[:], 1.0 / D, None, op0=ALU.mult)
                ffc = rowq[32:33, :]
                nc.vector.tensor_scalar(ffc[:], ff[:], mu[:], None, op0=ALU.subtract)
                ffsq = rowq[0:1, :]  # scratch, t1 dead
                vs = stp.tile([1, 1], F32, tag="vs")
                if use_ttr:
                    nc.vector.tensor_tensor_reduce(
                        out=ffsq[:], in0=ffc[:], in1=ffc[:], scale=1.0,
                        scalar=0.0, op0=ALU.mult, op1=ALU.add, accum_out=vs[:])
                else:
                    nc.vector.tensor_mul(ffsq[:], ffc[:], ffc[:])
                    nc.vector.reduce_sum(vs[:], ffsq[:], AX.X)
                stdv = stp.tile([1, 1], F32, tag="stdv")
                nc.scalar.activation(stdv[:], vs[:], AF.Sqrt, bias=eps_sb[0:1, :],
                                     scale=1.0 / D)
                rstd = stp.tile([1, 1], F32, tag="rstd")
                nc.vector.reciprocal(rstd[:], stdv[:])
                fused = rowq[0:1, :]  # must be base 0 for PE transpose
                nc.vector.tensor_mul(fused[:], ffc[:], lnffg_sb[:])
                nc.vector.tensor_scalar(fused[:], fused[:], rstd[:], None,
                                        op0=ALU.mult)
                nc.vector.tensor_add(fused[:], fused[:], lnffb_sb[:])

                # --- gate ---
                pfT = psS.tile([P, DC], F32, tag="ps")
                for jd in range(DC):
                    pe_transpose(pfT[:, jd:jd + 1],
                                 fused[:, jd * P:(jd + 1) * P], one11[:])
                fusedT_bf = scp.tile([P, DC], BF16, tag="fusedT")
                nc.scalar.copy(fusedT_bf[:], pfT[:])
                gpre = rowq[64:65, :]  # ff dead
                pgs = [psS.tile([1, 512], F32, tag="ps", name=f"pg{h2}")
                       for h2 in range(2)]
                for jd in range(DC):
                    if feat_bf16:
                        wg_c = wgp.tile([P, D], BF16, tag="wg",
                                        name=f"wgc{jd}")
                        nc.gpsimd.dma_start(wg_c[:], w_gatet_re[:, jd, :])
                    else:
                        wg_c = w_gatet_sb[:, jd, :]
                    for h2 in range(2):
                        nc.tensor.matmul(pgs[h2][:],
                                         lhsT=fusedT_bf[:, jd:jd + 1],
                                         rhs=wg_c[:, h2 * 512:(h2 + 1) * 512],
                                         start=(jd == 0), stop=(jd == DC - 1))
                for h2 in range(2):
                    nc.vector.tensor_add(gpre[:, h2 * 512:(h2 + 1) * 512],
                                         pgs[h2][:],
                                         b_gate_sb[:, h2 * 512:(h2 + 1) * 512])
                # sigmoid into PSUM so the fused*gate multiply has a PSUM
                # operand (exempt from the equal-base-partition rule)
                gps = [psS.tile([1, 512], F32, tag="ps", name=f"gps{h2}")
                       for h2 in range(2)]
                for h2 in range(2):
                    nc.scalar.activation(gps[h2][:],
                                         gpre[:, h2 * 512:(h2 + 1) * 512],
                                         AF.Sigmoid)
                c_row = rowq[0:1, :]  # in-place over fused (last use)
                for h2 in range(2):
                    nc.vector.tensor_mul(c_row[:, h2 * 512:(h2 + 1) * 512],
                                         fused[:, h2 * 512:(h2 + 1) * 512],
                                         gps[h2][:])
                for h2 in range(2):
                    pcb = psS.tile([P, 512], F32, tag="ps")
                    nc.tensor.matmul(pcb[:], lhsT=ones_1x128[:],
                                     rhs=c_row[:, h2 * 512:(h2 + 1) * 512],
                                     start=True, stop=True)
                    for hh in range(2):
                        nc.scalar.copy(
                            c_bcast[b][:, hh, h2 * 512:(h2 + 1) * 512], pcb[:])

            # ================= PASS 2: LN(x + c) =================
            def pass2(b, groups=None):
                G = 2  # pairs per stats batch: one Sqrt/recip per group
                for g in (range(NTP // G) if groups is None else groups):
                    xs, xas, os_ = [], [], []
                    mvg = stp.tile([P, G, 2, 2], F32, tag="mvg", bufs=2,
                                   name=f"mvg{b}{g}")
                    for j in range(G):
                        i = g * G + j
                        if feat_bf16 and b == 0:
                            x = xtiles[b][i]
                        else:
                            x = p2p.tile([P, 2, D], FDT, tag="x2", bufs=4,
                                         name=f"x2_{b}_{i}")
                            src = feat[i * 2 * P:(i + 1) * 2 * P, b,
                                       :].rearrange("(h p) d -> p h d", p=P)
                            nc.sync.dma_start(x[:], src)
                        xa = p2p.tile([P, 2, D], FDT if use_sq_accum else F32,
                                      tag="xa", bufs=G + 3)
                        nc.vector.tensor_add(xa[:], x[:], cb2[b][:])
                        stats = stp.tile([P, 2, 2, 6], F32, tag="bst",
                                         bufs=G + 1)
                        xa4 = xa.rearrange("p h (s f) -> p h s f", f=512)
                        for hh in range(2):
                            for si in range(2):
                                nc.vector.bn_stats(stats[:, hh, si, :],
                                                   xa4[:, hh, si, :])
                        for hh in range(2):
                            nc.vector.bn_aggr(mvg[:, j, hh, :],
                                              stats[:, hh, :, :])
                        xs.append(x)
                        xas.append(xa)
                    # batched rstd/-mu*rstd for the whole group (one ACT
                    # function load per group instead of per pair)
                    rstg = stp.tile([P, 3, G, 2], F32, tag="rstg", bufs=2,
                                    name=f"rstg{b}{g}")
                    mvv = mvg.rearrange("p g h t -> p (g h) t")
                    nc.scalar.activation(rstg[:, 0, :, :].rearrange(
                        "p g h -> p (g h)"), mvv[:, :, 1], AF.Sqrt,
                        bias=eps_sb[:])
                    nc.vector.reciprocal(rstg[:, 1, :, :].rearrange(
                        "p g h -> p (g h)"), rstg[:, 0, :, :].rearrange(
                        "p g h -> p (g h)"))
                    nc.vector.tensor_scalar(rstg[:, 2, :, :].rearrange(
                        "p g h -> p (g h)"), mvv[:, :, 0], -1.0, None,
                        op0=ALU.mult)
                    nc.vector.tensor_mul(rstg[:, 2, :, :].rearrange(
                        "p g h -> p (g h)"), rstg[:, 2, :, :].rearrange(
                        "p g h -> p (g h)"), rstg[:, 1, :, :].rearrange(
                        "p g h -> p (g h)"))
                    for j in range(G):
                        i = g * G + j
                        xa = xas[j]
                        o = p2p.tile([P, 2, D], ODT, tag="o", bufs=3)
                        dst = out[i * 2 * P:(i + 1) * 2 * P, b, :].rearrange(
                            "(h p) d -> p h d", p=P)
                        if use_sq_accum:
                            for hh in range(2):
                                nc.scalar.activation(
                                    o[:, hh, :], xa[:, hh, :], AF.Identity,
                                    bias=rstg[:, 2, j, hh:hh + 1],
                                    scale=rstg[:, 1, j, hh:hh + 1])
                        else:
                            for hh in range(2):
                                nc.vector.tensor_scalar(
                                    o[:, hh, :], xa[:, hh, :],
                                    mvg[:, j, hh, 0:1],
                                    rstg[:, 1, j, hh:hh + 1],
                                    op0=ALU.subtract, op1=ALU.mult)
                        if apply_ln_affine:
                            nc.vector.tensor_mul(o[:], o[:], g_bcast2[:])
                            nc.vector.tensor_add(o[:], o[:], b_bcast2[:])
                        if feat_bf16:
                            nc.scalar.dma_start(dst, o[:])
                        else:
                            nc.sync.dma_start(dst, o[:])

            def whole_kernel():
                if feat_bf16:
                    # pipeline: b0 loads -> SC(b0) (incl its DMAs early on
                    # the sync queue) ... b1 loads stream behind, b1 P1
                    # matmuls after SC(b0)'s PE work, pass2(b0) overlaps
                    # them, SC(b1) then pass2(b1).
                    pass1_loads(0)
                    pass1_mms(0)
                    f0, sw0 = sc_part1(0)
                    sc_part2(0, f0, sw0)
                    pass1_loads(1)
                    pass1_mms(1)
                    ngrp = NTP // 2  # pass2 G=2 groups per batch elem
                    pass2(0, groups=list(range(ngrp // 2)))
                    f1, sw1 = sc_part1(1)
                    pass2(0, groups=list(range(ngrp // 2, 3 * ngrp // 4)))
                    sc_part2(1, f1, sw1)
                    pass2(0, groups=list(range(3 * ngrp // 4, ngrp)))
                    pass2(1)
                else:
                    for b in range(BPC):
                        pass1_loads(b)
                        pass1_mms(b)
                        fb, swb = sc_part1(b)
                        sc_part2(b, fb, swb)
                        pass2(b)

            if reps > 1:
                with tc.For_i(0, reps, 1):
                    whole_kernel()
            else:
                whole_kernel()

            if os.environ.get("KERNEL_DEBUG_SIZES"):
                pools = [consts, wpool, bpool, scp, rows, p2p, stp]
                if feat_bf16:
                    pools += [cachep, wffp]
                else:
                    pools += [p1p]
                tot = 0
                for pl in pools:
                    sz = pl.current_size() / 128 / 1024
                    tot += sz
                    print(f"pool {pl.name}: {sz:.1f} KB/partition")
                print(f"TOTAL SBUF: {tot:.1f} KB/partition of "
                      f"{nc.SBUF_PARTITION_SIZE_BYTES/1024:.0f}")
                for pl in [psS, psC]:
                    print(f"pool {pl.name}: {pl.current_size()/128/2048:.1f} banks")

    nc.compile()
    return nc


_CACHE = {}


def _get_nc(apply_ln_affine: bool):
    key = (apply_ln_affine, FLAGS["feat_bf16"], FLAGS["use_ttr"],
           FLAGS["use_sq_accum"], FLAGS["use_pe_transpose"],
           FLAGS["out_bf16"])
    if key not in _CACHE:
        _CACHE[key] = _build(apply_ln_affine, *key[1:])
    return _CACHE[key]


def build_in_maps(inputs):
    features = np.asarray(inputs["features"], np.float32)
    f32 = lambda a: np.ascontiguousarray(np.asarray(a, np.float32))
    bf = lambda a: np.ascontiguousarray(
        np.asarray(a, np.float32).astype(ml_dtypes.bfloat16))

    b_rs2 = f32(inputs["b_rs2"])
    ln_g, ln_b = f32(inputs["ln_g"]), f32(inputs["ln_b"])
    dl = float(np.asarray(inputs["decay_local"]))
    dg = float(np.asarray(inputs["decay_global"]))
    apply_ln_affine = not (np.all(ln_g == 1.0) and np.all(ln_b == 0.0))

    pos = np.arange(L, dtype=np.float64)
    dlb = (np.log(dl ** (L - 1 - pos) + 1e-8) + float(b_rs2[0])).astype(np.float32)
    dgb = (np.log(dg ** (L - 1 - pos) + 1e-8) + float(b_rs2[0])).astype(np.float32)

    common = {
        "w_sp1t": bf(f32(inputs["w_sp1"]).T),
        "b_sp1t": f32(f32(inputs["b_sp1"]).reshape(HC, P).T),
        "w_sp2t": bf(f32(inputs["w_sp2"]).T),
        "b_sp2": f32(inputs["b_sp2"]).reshape(1, 3),
        "w_rs1t": bf(f32(inputs["w_rs1"]).T),
        "b_rs1": f32(inputs["b_rs1"]).reshape(1, H),
        "w_rs2r": f32(inputs["w_rs2"]).reshape(1, H),
        "dlb": dlb.reshape(1, L),
        "dgb": dgb.reshape(1, L),
        "w_fft": bf(f32(inputs["w_ff"]).T),
        "b_ff": f32(inputs["b_ff"]).reshape(1, D),
        "lnffg": f32(inputs["ln_ff_g"]).reshape(1, D),
        "lnffb": f32(inputs["ln_ff_b"]).reshape(1, D),
        "w_gatet": bf(f32(inputs["w_gate"]).T),
        "b_gate": f32(inputs["b_gate"]).reshape(1, D),
        "i32": np.eye(L, dtype=np.float32),
    }
    if FLAGS["feat_bf16"]:
        common["i32b"] = np.eye(L, dtype=np.float32).astype(ml_dtypes.bfloat16)
    if apply_ln_affine:
        common["lng"] = ln_g.reshape(1, D)
        common["lnb"] = ln_b.reshape(1, D)

    if FLAGS["feat_bf16"]:
        features_dev = features.astype(ml_dtypes.bfloat16)
    else:
        features_dev = features

    in_maps = []
    for c in range(NC):
        m = dict(common)
        m["features"] = np.ascontiguousarray(
            features_dev[:, c * BPC:(c + 1) * BPC, :])
        in_maps.append(m)
    return in_maps


def kernel(features, w_sp1, b_sp1, w_sp2, b_sp2, w_rs1, b_rs1, w_rs2, b_rs2,
           decay_local, decay_global, w_ff, b_ff, ln_ff_g, ln_ff_b,
           w_gate, b_gate, ln_g, ln_b, _trace=False):
    inputs = dict(features=features, w_sp1=w_sp1, b_sp1=b_sp1, w_sp2=w_sp2,
                  b_sp2=b_sp2, w_rs1=w_rs1, b_rs1=b_rs1, w_rs2=w_rs2,
                  b_rs2=b_rs2, decay_local=decay_local,
                  decay_global=decay_global, w_ff=w_ff, b_ff=b_ff,
                  ln_ff_g=ln_ff_g, ln_ff_b=ln_ff_b, w_gate=w_gate,
                  b_gate=b_gate, ln_g=ln_g, ln_b=ln_b)
    features = np.asarray(features, np.float32)
    ln_g_np = np.asarray(ln_g, np.float32)
    ln_b_np = np.asarray(ln_b, np.float32)
    apply_ln_affine = not (np.all(ln_g_np == 1.0) and np.all(ln_b_np == 0.0))
    nc = _get_nc(apply_ln_affine)
    in_maps = build_in_maps(inputs)

    res = run_bass_kernel_spmd(nc, in_maps, core_ids=list(range(NC)),
                               trace=_trace)
    output = np.concatenate([np.asarray(res.results[c]["out"], np.float32)
                             for c in range(NC)], axis=1)
    attention_weights = np.full((S, B), 1.0 / S, dtype=features.dtype)
    if _trace:
        kernel.last_exec_time_ns = res.exec_time_ns
        kernel.last_trace = res.instructions_and_trace
    return output, attention_weights


# revision 84
# speedup vs baseline: 1.0551x; 1.0551x over previous
"""Trainium2 Bass kernel for nn_AdaptiveSequenceProcessor.

Math (see reference):
  context  = mean_s(features)                               [B, D]
  span_w   = softmax(MLP_sp(context))                       [B, 3]
  feat_l   = relevance_pool(features[-32:],  context, 0.9)  [B, D]
  feat_g   = relevance_pool(features[::128], context, 0.8)  [B, D]
  fused    = LN(gelu(W_ff @ [feat_l*sw1 | feat_g*sw2]))     [B, D]
  gate     = sigmoid(W_g @ fused)  (constant over s!)       [B, D]
  out      = LN(features + fused*gate)                      [S, B, D]

Key structural fact: fused_expanded/gate are constant along the sequence
axis, so per batch element the heavy work is one mean over S (pass 1)
and one LN(features + c_b) sweep (pass 2); everything else is tiny.

Sharding: data-parallel over batch. 16 batch elements / 8 cores = 2 per
core, weights replicated, no collectives needed.

With FEAT_BF16: features are pre-cast to bf16 on host (halves the HBM
read), pass-1 tiles stay resident in SBUF and pass 2 reads them from
SBUF (no second HBM read). w_fft is streamed from DRAM per use to make
room for the cache.
"""

import os
import sys

sys.path.insert(0, "/opt/trn_rl_repo")

import numpy as np
import ml_dtypes

import concourse.bass as bass
import concourse.tile as tile
from concourse import bacc, mybir
from concourse.bass_utils import run_bass_kernel_spmd

F32 = mybir.dt.float32
BF16 = mybir.dt.bfloat16
AF = mybir.ActivationFunctionType
ALU = mybir.AluOpType
AX = mybir.AxisListType

S, B, D, H = 4096, 16, 1024, 512
P = 128          # SBUF partitions
NC = 8           # cores
BPC = B // NC    # batch per core = 2
NT = S // P      # 32 sequence tiles of 128
L = 32           # pool history length
DC = D // P      # 8 d-chunks of 128
HC = H // P      # 4 h-chunks of 128
LN_EPS = 1e-5
CACHE_SLOTS = 16   # pass-1 tile-pairs of b0 kept resident for pass 2
NTP = NT // 2    # pass-1/2 work in pairs of 128-row tiles: [128, 2048]

# Feature flags (module-level so test.py can flip them for experiments)
FLAGS = dict(
    feat_bf16=True,    # bf16 features input + SBUF cache + streamed w_fft
    use_ttr=False,     # tensor_tensor_reduce — CRASHES HW (exec unit
                       # unrecoverable), never enable
    use_sq_accum=True,  # ACT Square with accum_out + Identity bias/scale apply
    use_pe_transpose=False,  # is_transpose matmuls instead of identity matmul
    out_bf16=True,     # write output as bf16, upcast on host (halves write BW)
)


def _build(apply_ln_affine: bool, feat_bf16: bool, use_ttr: bool,
           use_sq_accum: bool, use_pe_transpose: bool, out_bf16: bool = True,
           reps: int = 1):
    """Build the per-core Bass graph. Returns compiled Bacc.

    reps > 1 wraps the whole kernel in a tc.For_i hardware loop — used
    only by the timing harness to amortize per-call dispatch overhead.
    """
    nc = bacc.Bacc("TRN2", target_bir_lowering=False, debug=False,
                   num_devices=NC)

    FDT = BF16 if feat_bf16 else F32

    feat = nc.declare_dram_parameter("features", [S, BPC, D], FDT, isOutput=False)
    w_sp1t = nc.declare_dram_parameter("w_sp1t", [D, H], BF16, isOutput=False)
    b_sp1t = nc.declare_dram_parameter("b_sp1t", [P, HC], F32, isOutput=False)
    w_sp2t = nc.declare_dram_parameter("w_sp2t", [H, 3], BF16, isOutput=False)
    b_sp2 = nc.declare_dram_parameter("b_sp2", [1, 3], F32, isOutput=False)
    w_rs1t = nc.declare_dram_parameter("w_rs1t", [2 * D, H], BF16, isOutput=False)
    b_rs1 = nc.declare_dram_parameter("b_rs1", [1, H], F32, isOutput=False)
    w_rs2r = nc.declare_dram_parameter("w_rs2r", [1, H], F32, isOutput=False)
    dlb = nc.declare_dram_parameter("dlb", [1, L], F32, isOutput=False)
    dgb = nc.declare_dram_parameter("dgb", [1, L], F32, isOutput=False)
    w_fft = nc.declare_dram_parameter("w_fft", [2 * D, D], BF16, isOutput=False)
    b_ff = nc.declare_dram_parameter("b_ff", [1, D], F32, isOutput=False)
    lnffg = nc.declare_dram_parameter("lnffg", [1, D], F32, isOutput=False)
    lnffb = nc.declare_dram_parameter("lnffb", [1, D], F32, isOutput=False)
    w_gatet = nc.declare_dram_parameter("w_gatet", [D, D], BF16, isOutput=False)
    b_gate = nc.declare_dram_parameter("b_gate", [1, D], F32, isOutput=False)
    i32d = nc.declare_dram_parameter("i32", [L, L], F32, isOutput=False)
    if feat_bf16:
        i32bd = nc.declare_dram_parameter("i32b", [L, L], BF16, isOutput=False)
    if apply_ln_affine:
        lng = nc.declare_dram_parameter("lng", [1, D], F32, isOutput=False)
        lnb = nc.declare_dram_parameter("lnb", [1, D], F32, isOutput=False)
    ODT = BF16 if out_bf16 else F32
    out = nc.declare_dram_parameter("out", [S, BPC, D], ODT, isOutput=True)

    w_fft_re = w_fft.ap().rearrange("(c p) n -> p c n", p=P)
    w_gatet_re = w_gatet.ap().rearrange("(c p) n -> p c n", p=P)

    with tile.TileContext(nc) as tc:
        from contextlib import ExitStack
        with ExitStack() as ctx:
            consts = ctx.enter_context(tc.tile_pool(name="consts", bufs=1))
            wpool = ctx.enter_context(tc.tile_pool(name="weights", bufs=1))
            bpool = ctx.enter_context(tc.tile_pool(name="perb", bufs=1))
            scp = ctx.enter_context(tc.tile_pool(name="sc", bufs=1))
            rows = ctx.enter_context(tc.tile_pool(name="rows", bufs=1))
            if feat_bf16:
                cachep = ctx.enter_context(tc.tile_pool(name="cache", bufs=CACHE_SLOTS))
                wffp = ctx.enter_context(tc.tile_pool(name="wff", bufs=3))
                wgp = ctx.enter_context(tc.tile_pool(name="wg", bufs=3))
                p1sp = ctx.enter_context(tc.tile_pool(name="p1s", bufs=3))
            else:
                p1p = ctx.enter_context(tc.tile_pool(name="p1", bufs=4))
            p2p = ctx.enter_context(tc.tile_pool(name="p2", bufs=2))
            stp = ctx.enter_context(tc.tile_pool(name="stats", bufs=3))
            psS = ctx.enter_context(tc.tile_pool(name="psS", bufs=5, space="PSUM"))
            psC = ctx.enter_context(tc.tile_pool(name="psC", bufs=1, space="PSUM"))

            # ---- constants ----
            ones128 = consts.tile([P, 1], FDT)
            nc.vector.memset(ones128[:], 1.0)
            one11 = consts.tile([1, 1], F32)
            nc.vector.memset(one11[:], 1.0)
            ones_1x32_bf = consts.tile([1, L], BF16)
            nc.vector.memset(ones_1x32_bf[:], 1.0)
            ones_1x128 = consts.tile([1, P], F32)
            nc.vector.memset(ones_1x128[:], 1.0)
            i32sb = consts.tile([L, L], F32)
            nc.scalar.dma_start(i32sb[:], i32d[:])
            if feat_bf16:
                i32bsb = consts.tile([L, L], BF16)
                nc.scalar.dma_start(i32bsb[:], i32bd[:])
            eps_sb = consts.tile([P, 1], F32)
            nc.vector.memset(eps_sb[:], LN_EPS)

            # ---- weights to SBUF (gpsimd/SWDGE queue, ordered by need;
            # the sync queue stays free for feature loads) ----
            w_sp1t_sb = wpool.tile([P, DC, H], BF16)
            nc.scalar.dma_start(w_sp1t_sb[:], w_sp1t.ap().rearrange("(c p) n -> p c n", p=P))
            w_rs1t_sb = wpool.tile([P, 2 * DC, H], BF16)
            nc.scalar.dma_start(w_rs1t_sb[:], w_rs1t.ap().rearrange("(c p) n -> p c n", p=P))
            b_sp1t_sb = wpool.tile([P, HC], F32)
            nc.scalar.dma_start(b_sp1t_sb[:], b_sp1t[:])
            w_sp2t_sb = wpool.tile([P, HC, 3], BF16)
            nc.scalar.dma_start(w_sp2t_sb[:], w_sp2t.ap().rearrange("(c p) n -> p c n", p=P))
            if not feat_bf16:
                w_fft_sb = wpool.tile([P, 2 * DC, D], BF16)
                nc.scalar.dma_start(w_fft_sb[:], w_fft_re)
                w_gatet_sb = wpool.tile([P, DC, D], BF16)
                nc.scalar.dma_start(w_gatet_sb[:], w_gatet.ap().rearrange("(c p) n -> p c n", p=P))

            b_sp2_sb = consts.tile([1, 3], F32)
            nc.scalar.dma_start(b_sp2_sb[:], b_sp2[:])
            b_rs1_sb = consts.tile([1, H], F32)
            nc.scalar.dma_start(b_rs1_sb[:], b_rs1[:])
            w_rs2r_sb = consts.tile([1, H], F32)
            nc.scalar.dma_start(w_rs2r_sb[:], w_rs2r[:])
            dlb_sb = consts.tile([1, L], F32)
            nc.scalar.dma_start(dlb_sb[:], dlb[:])
            dgb_sb = consts.tile([1, L], F32)
            nc.scalar.dma_start(dgb_sb[:], dgb[:])
            # four [1,D] const rows packed at quadrant partitions of one
            # tile (engine operands accept start partitions 0/32/64/96)
            constrows = consts.tile([P, 2, D], F32)
            nc.scalar.dma_start(constrows[0:1, 0, :], b_ff[:])
            nc.scalar.dma_start(constrows[0:1, 1, :], lnffb[:])
            nc.scalar.dma_start(constrows[32:33, 0, :], lnffg[:])
            nc.scalar.dma_start(constrows[64:65, 0, :], b_gate[:])
            b_ff_sb = constrows[0:1, 0, :]
            lnffb_sb = constrows[0:1, 1, :]
            lnffg_sb = constrows[32:33, 0, :]
            b_gate_sb = constrows[64:65, 0, :]

            # w_rs2 broadcast to 32 partitions (one-time)
            pw32 = psS.tile([L, H], F32, tag="ps")
            ones_1x32_f = consts.tile([1, L], F32)
            nc.vector.memset(ones_1x32_f[:], 1.0)
            nc.tensor.matmul(pw32[:], lhsT=ones_1x32_f[:], rhs=w_rs2r_sb[:],
                             start=True, stop=True)
            w_rs2b = consts.tile([L, H], F32)
            nc.scalar.copy(w_rs2b[:], pw32[:])
            # w_rs2 as [128, HC] bf16 columns (for the transposed-h scores)
            pwc4 = psS.tile([P, HC], F32, tag="ps")
            for jh in range(HC):
                nc.tensor.matmul(pwc4[:, jh:jh + 1],
                                 lhsT=w_rs2r_sb[:, jh * P:(jh + 1) * P],
                                 rhs=one11[:], start=True, stop=True)
            w_rs2c_sb = consts.tile([P, HC], BF16)
            nc.scalar.copy(w_rs2c_sb[:], pwc4[:])

            if apply_ln_affine:
                lng_sb = consts.tile([1, D], F32)
                nc.scalar.dma_start(lng_sb[:], lng[:])
                lnb_sb = consts.tile([1, D], F32)
                nc.scalar.dma_start(lnb_sb[:], lnb[:])
                g_bcast2 = bpool.tile([P, 2, D], F32)
                b_bcast2 = bpool.tile([P, 2, D], F32)
                for h2 in range(2):
                    pgb = psS.tile([P, 512], F32, tag="ps")
                    nc.tensor.matmul(pgb[:], lhsT=ones_1x128[:],
                                     rhs=lng_sb[:, h2 * 512:(h2 + 1) * 512],
                                     start=True, stop=True)
                    pbb = psS.tile([P, 512], F32, tag="ps")
                    nc.tensor.matmul(pbb[:], lhsT=ones_1x128[:],
                                     rhs=lnb_sb[:, h2 * 512:(h2 + 1) * 512],
                                     start=True, stop=True)
                    for hh in range(2):
                        nc.scalar.copy(g_bcast2[:, hh, h2 * 512:(h2 + 1) * 512],
                                       pgb[:])
                        nc.scalar.copy(b_bcast2[:, hh, h2 * 512:(h2 + 1) * 512],
                                       pbb[:])

            CBD = BF16 if feat_bf16 else F32
            c_bcast = [bpool.tile([P, 2, D], CBD, tag=f"cb{b}",
                       name=f"c_bcast{b}") for b in range(BPC)]
            cb2 = c_bcast
            ctxT_bf = [bpool.tile([P, DC], BF16, tag=f"ctxT{b}", name=f"ctxT{b}")
                       for b in range(BPC)]

            def pe_transpose(out_psum, in_sb, ident):
                if use_pe_transpose:
                    nc.tensor.transpose(out_psum, in_sb, ident)
                else:
                    nc.tensor.matmul(out_psum, lhsT=in_sb, rhs=ident,
                                     start=True, stop=True)

            # identity for transposing FDT-typed tiles
            i32f = i32bsb if feat_bf16 else i32sb

            # ================= PASS 1: context mean =================
            xtiles = [[None] * NT for _ in range(BPC)]

            def pass1_loads(b):
                # b=0 tiles land in the resident cache (read again by pass
                # 2); b=1 tiles stream through a small pool (pass 2
                # re-reads from HBM) so b1's pass 1 is decoupled from b0's
                # pass 2 progress.
                for i in range(NTP):
                    if feat_bf16:
                        if b == 0:
                            x = cachep.tile([P, 2, D], FDT, tag="xc",
                                            name=f"x_{b}_{i}")
                        else:
                            x = p1sp.tile([P, 2, D], FDT, tag="xs",
                                          name=f"x_{b}_{i}")
                    else:
                        x = p1p.tile([P, 2, D], FDT, tag="x", name=f"x_{b}_{i}")
                    xtiles[b][i] = x
                    src = feat[i * 2 * P:(i + 1) * 2 * P, b, :].rearrange(
                        "(h p) d -> p h d", p=P)
                    nc.sync.dma_start(x[:], src)

            def pass1_mms(b):
                pc = psC.tile([1, D], F32, tag="pc")  # 2 banks
                for i in range(NTP):
                    x = xtiles[b][i]
                    for hh in range(2):
                        for h2 in range(2):
                            nc.tensor.matmul(
                                pc[:, h2 * 512:(h2 + 1) * 512],
                                lhsT=ones128[:],
                                rhs=x[:, hh, h2 * 512:(h2 + 1) * 512],
                                start=(i == 0 and hh == 0),
                                stop=(i == NTP - 1 and hh == 1))
                ctx_row = scp.tile([1, D], F32, tag="ctxrow")
                nc.scalar.mul(ctx_row[:], pc[:], 1.0 / S)
                pct = psS.tile([P, DC], F32, tag="ps")
                for j in range(DC):
                    pe_transpose(pct[:, j:j + 1],
                                 ctx_row[:, j * P:(j + 1) * P], one11[:])
                nc.scalar.copy(ctxT_bf[b][:], pct[:])

            # ================= SMALL COMPUTE per b =================
            def sc_part1(b):
                # --- phase A: ctx-independent hist loads + transposes ---
                hists, histTs = [], []
                for pi in range(2):
                    hist = scp.tile([L, D], FDT, tag=f"hist{pi}",
                                    name=f"hist{pi}")
                    if pi == 0:
                        nc.sync.dma_start(hist[:], feat[S - L:S, b, :])
                    else:
                        nc.sync.dma_start(hist[:], feat[0:S:P, b, :])
                    hists.append(hist)
                for pi in range(2):
                    histT_bf = scp.tile([P, DC, L], BF16, tag=f"histT{pi}",
                                        name=f"histT{pi}")
                    for jd in range(DC):
                        pt = psS.tile([P, L], F32, tag="ps")
                        pe_transpose(pt[:], hists[pi][:, jd * P:(jd + 1) * P],
                                     i32f[:])
                        nc.scalar.copy(histT_bf[:, jd, :], pt[:])
                    histTs.append(histT_bf)
                    if not feat_bf16:
                        hist_bf = scp.tile([L, D], BF16, tag=f"histbf{pi}",
                                           name=f"histbf{pi}")
                        nc.scalar.copy(hist_bf[:], hists[pi][:])
                        hists[pi] = hist_bf

                # --- span MLP (needs ctx) ---
                psp = psS.tile([P, HC], F32, tag="ps")
                for jh in range(HC):
                    for jd in range(DC):
                        nc.tensor.matmul(
                            psp[:, jh:jh + 1],
                            lhsT=w_sp1t_sb[:, jd, jh * P:(jh + 1) * P],
                            rhs=ctxT_bf[b][:, jd:jd + 1],
                            start=(jd == 0), stop=(jd == DC - 1))
                sp_hT = scp.tile([P, HC], BF16, tag="sphT")
                for jh in range(HC):
                    nc.scalar.activation(sp_hT[:, jh:jh + 1], psp[:, jh:jh + 1],
                                         AF.Gelu, bias=b_sp1t_sb[:, jh:jh + 1])
                psl = psS.tile([1, 3], F32, tag="ps")
                for jh in range(HC):
                    nc.tensor.matmul(psl[:], lhsT=sp_hT[:, jh:jh + 1],
                                     rhs=w_sp2t_sb[:, jh, :],
                                     start=(jh == 0), stop=(jh == HC - 1))
                # softmax over 3 logits (values are O(1): no max-subtract)
                e3 = rows.tile([1, 3], F32, tag="e3")
                nc.vector.tensor_add(e3[:], psl[:], b_sp2_sb[:])
                nc.scalar.activation(e3[:], e3[:], AF.Exp)
                z3 = stp.tile([1, 1], F32, tag="z3")
                nc.vector.reduce_sum(z3[:], e3[:], AX.X)
                rz3 = stp.tile([1, 1], F32, tag="rz3")
                nc.vector.reciprocal(rz3[:], z3[:])
                sw = rows.tile([1, 3], F32, tag="sw")
                nc.vector.tensor_scalar(sw[:], e3[:], rz3[:], None, op0=ALU.mult)

                # --- ctx part of relevance-MLP (shared by both pools) ---
                pcr = psS.tile([1, H], F32, tag="ps")
                for jd in range(DC):
                    nc.tensor.matmul(pcr[:], lhsT=ctxT_bf[b][:, jd:jd + 1],
                                     rhs=w_rs1t_sb[:, DC + jd, :],
                                     start=(jd == 0), stop=(jd == DC - 1))
                ctx_rs_bf = scp.tile([1, H], BF16, tag="ctxrs")
                nc.vector.tensor_add(ctx_rs_bf[:], pcr[:], b_rs1_sb[:])

                # --- two relevance pools, phase-interleaved ---
                srows, wcols = [], []
                # hidden layer computed TRANSPOSED (h^T[hh, l]) so the
                # score reduction over hh happens on PE (partition
                # contraction) instead of a DVE mul+reduce + transpose.
                hTs = []
                for pi in range(2):
                    phT = psS.tile([P, HC, L], F32, tag="ps",
                                   name=f"phT{pi}")
                    for jh in range(HC):
                        for jd in range(DC):
                            nc.tensor.matmul(
                                phT[:, jh, :],
                                lhsT=w_rs1t_sb[:, jd, jh * P:(jh + 1) * P],
                                rhs=histTs[pi][:, jd, :],
                                start=(jd == 0), stop=False)
                        nc.tensor.matmul(
                            phT[:, jh, :],
                            lhsT=ctx_rs_bf[:, jh * P:(jh + 1) * P],
                            rhs=ones_1x32_bf[:], start=False, stop=True)
                    hT = scp.tile([P, HC, L], BF16, tag=f"hT{pi}",
                                  name=f"hT{pi}")
                    nc.scalar.activation(hT[:], phT[:], AF.Gelu)
                    hTs.append(hT)
                for pi, dbias in enumerate([dlb_sb, dgb_sb]):
                    psc = psS.tile([1, L], F32, tag="ps", name=f"psc{pi}")
                    for jh in range(HC):
                        nc.tensor.matmul(psc[:], lhsT=w_rs2c_sb[:, jh:jh + 1],
                                         rhs=hTs[pi][:, jh, :],
                                         start=(jh == 0), stop=(jh == HC - 1))
                    # scores are O(1)-ish and the decay bias is <= 0: exp is
                    # safe without max-subtraction.
                    e32 = rows.tile([1, L], F32, tag=f"e32{pi}",
                                    name=f"e32{pi}")
                    nc.vector.tensor_add(e32[:], psc[:], dbias[:])
                    nc.scalar.activation(e32[:], e32[:], AF.Exp)
                    z32 = stp.tile([1, 1], F32, tag=f"z32{pi}",
                                   name=f"z32{pi}")
                    nc.vector.reduce_sum(z32[:], e32[:], AX.X)
                    rz32 = stp.tile([1, 1], F32, tag=f"rz32{pi}",
                                    name=f"rz32{pi}")
                    nc.vector.reciprocal(rz32[:], z32[:])
                    wrow = rows.tile([1, L], F32, tag=f"wrow{pi}",
                                     name=f"wrow{pi}")
                    nc.vector.tensor_scalar(wrow[:], e32[:], rz32[:], None,
                                            op0=ALU.mult)
                    srows.append(wrow)
                featT = []
                for pi in range(2):
                    pwc = psS.tile([L, 1], F32, tag="ps", name=f"pwc{pi}")
                    pe_transpose(pwc[:], srows[pi][:], one11[:])
                    wcol_bf = stp.tile([L, 1], BF16, tag=f"wcol{pi}",
                                       name=f"wcol{pi}")
                    nc.scalar.copy(wcol_bf[:], pwc[:])
                    wcols.append(wcol_bf)
                for pi in range(2):
                    pft = psS.tile([P, DC], F32, tag="ps", name=f"pft{pi}")
                    for jd in range(DC):
                        nc.tensor.matmul(pft[:, jd:jd + 1],
                                         lhsT=hists[pi][:, jd * P:(jd + 1) * P],
                                         rhs=wcols[pi][:], start=True, stop=True)
                    fT = scp.tile([P, DC], BF16, tag=f"featT{pi}",
                                  name=f"featT{pi}")
                    nc.scalar.copy(fT[:], pft[:])
                    featT.append(fT)
                return featT, sw

            def sc_part2(b, featT, sw):
                # --- fusion ff ---
                pAB = []
                for pi in range(2):
                    pAh = [psS.tile([1, 512], F32, tag="ps", name=f"pA{pi}{h2}")
                           for h2 in range(2)]
                    for jd in range(DC):
                        if feat_bf16:
                            wff_c = wffp.tile([P, D], BF16, tag="wff",
                                              name=f"wffc{pi}{jd}")
                            nc.gpsimd.dma_start(wff_c[:],
                                                w_fft_re[:, pi * DC + jd, :])
                        else:
                            wff_c = w_fft_sb[:, pi * DC + jd, :]
                        for h2 in range(2):
                            nc.tensor.matmul(
                                pAh[h2][:], lhsT=featT[pi][:, jd:jd + 1],
                                rhs=wff_c[:, h2 * 512:(h2 + 1) * 512],
                                start=(jd == 0), stop=(jd == DC - 1))
                    pAB.append(pAh)
                rowq = scp.tile([P, D], F32, tag="rowq")
                t1 = rowq[0:1, :]
                for h2 in range(2):
                    nc.vector.tensor_scalar(t1[:, h2 * 512:(h2 + 1) * 512],
                                            pAB[0][h2][:], sw[:, 1:2], None,
                                            op0=ALU.mult)
                    # scale the global half in PSUM, then accumulate from
                    # PSUM (PSUM operand is exempt from the equal-base rule)
                    nc.vector.tensor_scalar(pAB[1][h2][:], pAB[1][h2][:],
                                            sw[:, 2:3], None, op0=ALU.mult)
                    nc.vector.tensor_add(t1[:, h2 * 512:(h2 + 1) * 512],
                                         pAB[1][h2][:],
                                         t1[:, h2 * 512:(h2 + 1) * 512])
                nc.vector.tensor_add(t1[:], t1[:], b_ff_sb[:])
                ff = rowq[64:65, :]  # q2
                nc.scalar.activation(ff[:], t1[:], AF.Gelu)
                # LN over free axis
                smu = stp.tile([1, 1], F32, tag="smu")
                nc.vector.reduce_sum(smu[:], ff[:], AX.X)
                mu = stp.tile([1, 1], F32, tag="mu")
                nc.vector.tensor_scalar(mu[:], smu[:], 1.0 / D, None, op0=ALU.mult)
                ffc = rowq[32:33, :]
                nc.vector.tensor_scalar(ffc[:], ff[:], mu[:], None, op0=ALU.subtract)
                ffsq = rowq[0:1, :]  # scratch, t1 dead
                vs = stp.tile([1, 1], F32, tag="vs")
                if use_ttr:
                    nc.vector.tensor_tensor_reduce(
                        out=ffsq[:], in0=ffc[:], in1=ffc[:], scale=1.0,
                        scalar=0.0, op0=ALU.mult, op1=ALU.add, accum_out=vs[:])
                else:
                    nc.vector.tensor_mul(ffsq[:], ffc[:], ffc[:])
                    nc.vector.reduce_sum(vs[:], ffsq[:], AX.X)
                stdv = stp.tile([1, 1], F32, tag="stdv")
                nc.scalar.activation(stdv[:], vs[:], AF.Sqrt, bias=eps_sb[0:1, :],
                                     scale=1.0 / D)
                rstd = stp.tile([1, 1], F32, tag="rstd")
                nc.vector.reciprocal(rstd[:], stdv[:])
                fused = rowq[0:1, :]  # must be base 0 for PE transpose
                nc.vector.tensor_mul(fused[:], ffc[:], lnffg_sb[:])
                nc.vector.tensor_scalar(fused[:], fused[:], rstd[:], None,
                                        op0=ALU.mult)
                nc.vector.tensor_add(fused[:], fused[:], lnffb_sb[:])

                # --- gate ---
                pfT = psS.tile([P, DC], F32, tag="ps")
                for jd in range(DC):
                    pe_transpose(pfT[:, jd:jd + 1],
                                 fused[:, jd * P:(jd + 1) * P], one11[:])
                fusedT_bf = scp.tile([P, DC], BF16, tag="fusedT")
                nc.scalar.copy(fusedT_bf[:], pfT[:])
                gpre = rowq[64:65, :]  # ff dead
                pgs = [psS.tile([1, 512], F32, tag="ps", name=f"pg{h2}")
                       for h2 in range(2)]
                for jd in range(DC):
                    if feat_bf16:
                        wg_c = wgp.tile([P, D], BF16, tag="wg",
                                        name=f"wgc{jd}")
                        nc.gpsimd.dma_start(wg_c[:], w_gatet_re[:, jd, :])
                    else:
                        wg_c = w_gatet_sb[:, jd, :]
                    for h2 in range(2):
                        nc.tensor.matmul(pgs[h2][:],
                                         lhsT=fusedT_bf[:, jd:jd + 1],
                                         rhs=wg_c[:, h2 * 512:(h2 + 1) * 512],
                                         start=(jd == 0), stop=(jd == DC - 1))
                for h2 in range(2):
                    nc.vector.tensor_add(gpre[:, h2 * 512:(h2 + 1) * 512],
                                         pgs[h2][:],
                                         b_gate_sb[:, h2 * 512:(h2 + 1) * 512])
                # sigmoid into PSUM so the fused*gate multiply has a PSUM
                # operand (exempt from the equal-base-partition rule)
                gps = [psS.tile([1, 512], F32, tag="ps", name=f"gps{h2}")
                       for h2 in range(2)]
                for h2 in range(2):
                    nc.scalar.activation(gps[h2][:],
                                         gpre[:, h2 * 512:(h2 + 1) * 512],
                                         AF.Sigmoid)
                c_row = rowq[0:1, :]  # in-place over fused (last use)
                for h2 in range(2):
                    nc.vector.tensor_mul(c_row[:, h2 * 512:(h2 + 1) * 512],
                                         fused[:, h2 * 512:(h2 + 1) * 512],
                                         gps[h2][:])
                for h2 in range(2):
                    pcb = psS.tile([P, 512], F32, tag="ps")
                    nc.tensor.matmul(pcb[:], lhsT=ones_1x128[:],
                                     rhs=c_row[:, h2 * 512:(h2 + 1) * 512],
                                     start=True, stop=True)
                    for hh in range(2):
                        nc.scalar.copy(
                            c_bcast[b][:, hh, h2 * 512:(h2 + 1) * 512], pcb[:])

            # ================= PASS 2: LN(x + c) =================
            def pass2(b, groups=None):
                G = 2  # pairs per stats batch: one Sqrt/recip per group
                for g in (range(NTP // G) if groups is None else groups):
                    xs, xas, os_ = [], [], []
                    mvg = stp.tile([P, G, 2, 2], F32, tag="mvg", bufs=2,
                                   name=f"mvg{b}{g}")
                    for j in range(G):
                        i = g * G + j
                        if feat_bf16 and b == 0:
                            x = xtiles[b][i]
                        else:
                            x = p2p.tile([P, 2, D], FDT, tag="x2", bufs=4,
                                         name=f"x2_{b}_{i}")
                            src = feat[i * 2 * P:(i + 1) * 2 * P, b,
                                       :].rearrange("(h p) d -> p h d", p=P)
                            nc.sync.dma_start(x[:], src)
                        xa = p2p.tile([P, 2, D], FDT if use_sq_accum else F32,
                                      tag="xa", bufs=G + 3)
                        nc.vector.tensor_add(xa[:], x[:], cb2[b][:])
                        stats = stp.tile([P, 2, 2, 6], F32, tag="bst",
                                         bufs=G + 1)
                        xa4 = xa.rearrange("p h (s f) -> p h s f", f=512)
                        for hh in range(2):
                            for si in range(2):
                                nc.vector.bn_stats(stats[:, hh, si, :],
                                                   xa4[:, hh, si, :])
                        for hh in range(2):
                            nc.vector.bn_aggr(mvg[:, j, hh, :],
                                              stats[:, hh, :, :])
                        xs.append(x)
                        xas.append(xa)
                    # batched rstd/-mu*rstd for the whole group (one ACT
                    # function load per group instead of per pair)
                    rstg = stp.tile([P, 3, G, 2], F32, tag="rstg", bufs=2,
                                    name=f"rstg{b}{g}")
                    mvv = mvg.rearrange("p g h t -> p (g h) t")
                    nc.scalar.activation(rstg[:, 0, :, :].rearrange(
                        "p g h -> p (g h)"), mvv[:, :, 1], AF.Sqrt,
                        bias=eps_sb[:])
                    nc.vector.reciprocal(rstg[:, 1, :, :].rearrange(
                        "p g h -> p (g h)"), rstg[:, 0, :, :].rearrange(
                        "p g h -> p (g h)"))
                    nc.vector.tensor_scalar(rstg[:, 2, :, :].rearrange(
                        "p g h -> p (g h)"), mvv[:, :, 0], -1.0, None,
                        op0=ALU.mult)
                    nc.vector.tensor_mul(rstg[:, 2, :, :].rearrange(
                        "p g h -> p (g h)"), rstg[:, 2, :, :].rearrange(
                        "p g h -> p (g h)"), rstg[:, 1, :, :].rearrange(
                        "p g h -> p (g h)"))
                    for j in range(G):
                        i = g * G + j
                        xa = xas[j]
                        o = p2p.tile([P, 2, D], ODT, tag="o", bufs=3)
                        dst = out[i * 2 * P:(i + 1) * 2 * P, b, :].rearrange(
                            "(h p) d -> p h d", p=P)
                        if use_sq_accum:
                            for hh in range(2):
                                nc.scalar.activation(
                                    o[:, hh, :], xa[:, hh, :], AF.Identity,
                                    bias=rstg[:, 2, j, hh:hh + 1],
                                    scale=rstg[:, 1, j, hh:hh + 1])
                        else:
                            for hh in range(2):
                                nc.vector.tensor_scalar(
                                    o[:, hh, :], xa[:, hh, :],
                                    mvg[:, j, hh, 0:1],
                                    rstg[:, 1, j, hh:hh + 1],
                                    op0=ALU.subtract, op1=ALU.mult)
                        if apply_ln_affine:
                            nc.vector.tensor_mul(o[:], o[:], g_bcast2[:])
                            nc.vector.tensor_add(o[:], o[:], b_bcast2[:])
                        if feat_bf16:
                            nc.scalar.dma_start(dst, o[:])
                        else:
                            nc.sync.dma_start(dst, o[:])

            def whole_kernel():
                if feat_bf16:
                    # pipeline: b0 loads -> SC(b0) (incl its DMAs early on
                    # the sync queue) ... b1 loads stream behind, b1 P1
                    # matmuls after SC(b0)'s PE work, pass2(b0) overlaps
                    # them, SC(b1) then pass2(b1).
                    pass1_loads(0)
                    pass1_mms(0)
                    f0, sw0 = sc_part1(0)
                    sc_part2(0, f0, sw0)
                    pass1_loads(1)
                    pass1_mms(1)
                    ngrp = NTP // 2  # pass2 G=2 groups per batch elem
                    f1, sw1 = sc_part1(1)
                    sc_part2(1, f1, sw1)
                    pass2(0)
                    pass2(1)
                else:
                    for b in range(BPC):
                        pass1_loads(b)
                        pass1_mms(b)
                        fb, swb = sc_part1(b)
                        sc_part2(b, fb, swb)
                        pass2(b)

            if reps > 1:
                with tc.For_i(0, reps, 1):
                    whole_kernel()
            else:
                whole_kernel()

            if os.environ.get("KERNEL_DEBUG_SIZES"):
                pools = [consts, wpool, bpool, scp, rows, p2p, stp]
                if feat_bf16:
                    pools += [cachep, wffp]
                else:
                    pools += [p1p]
                tot = 0
                for pl in pools:
                    sz = pl.current_size() / 128 / 1024
                    tot += sz
                    print(f"pool {pl.name}: {sz:.1f} KB/partition")
                print(f"TOTAL SBUF: {tot:.1f} KB/partition of "
                      f"{nc.SBUF_PARTITION_SIZE_BYTES/1024:.0f}")
                for pl in [psS, psC]:
                    print(f"pool {pl.name}: {pl.current_size()/128/2048:.1f} banks")

    nc.compile()
    return nc


_CACHE = {}


def _get_nc(apply_ln_affine: bool):
    key = (apply_ln_affine, FLAGS["feat_bf16"], FLAGS["use_ttr"],
           FLAGS["use_sq_accum"], FLAGS["use_pe_transpose"],
           FLAGS["out_bf16"])
    if key not in _CACHE:
        _CACHE[key] = _build(apply_ln_affine, *key[1:])
    return _CACHE[key]


def build_in_maps(inputs):
    features = np.asarray(inputs["features"], np.float32)
    f32 = lambda a: np.ascontiguousarray(np.asarray(a, np.float32))
    bf = lambda a: np.ascontiguousarray(
        np.asarray(a, np.float32).astype(ml_dtypes.bfloat16))

    b_rs2 = f32(inputs["b_rs2"])
    ln_g, ln_b = f32(inputs["ln_g"]), f32(inputs["ln_b"])
    dl = float(np.asarray(inputs["decay_local"]))
    dg = float(np.asarray(inputs["decay_global"]))
    apply_ln_affine = not (np.all(ln_g == 1.0) and np.all(ln_b == 0.0))

    pos = np.arange(L, dtype=np.float64)
    dlb = (np.log(dl ** (L - 1 - pos) + 1e-8) + float(b_rs2[0])).astype(np.float32)
    dgb = (np.log(dg ** (L - 1 - pos) + 1e-8) + float(b_rs2[0])).astype(np.float32)

    common = {
        "w_sp1t": bf(f32(inputs["w_sp1"]).T),
        "b_sp1t": f32(f32(inputs["b_sp1"]).reshape(HC, P).T),
        "w_sp2t": bf(f32(inputs["w_sp2"]).T),
        "b_sp2": f32(inputs["b_sp2"]).reshape(1, 3),
        "w_rs1t": bf(f32(inputs["w_rs1"]).T),
        "b_rs1": f32(inputs["b_rs1"]).reshape(1, H),
        "w_rs2r": f32(inputs["w_rs2"]).reshape(1, H),
        "dlb": dlb.reshape(1, L),
        "dgb": dgb.reshape(1, L),
        "w_fft": bf(f32(inputs["w_ff"]).T),
        "b_ff": f32(inputs["b_ff"]).reshape(1, D),
        "lnffg": f32(inputs["ln_ff_g"]).reshape(1, D),
        "lnffb": f32(inputs["ln_ff_b"]).reshape(1, D),
        "w_gatet": bf(f32(inputs["w_gate"]).T),
        "b_gate": f32(inputs["b_gate"]).reshape(1, D),
        "i32": np.eye(L, dtype=np.float32),
    }
    if FLAGS["feat_bf16"]:
        common["i32b"] = np.eye(L, dtype=np.float32).astype(ml_dtypes.bfloat16)
    if apply_ln_affine:
        common["lng"] = ln_g.reshape(1, D)
        common["lnb"] = ln_b.reshape(1, D)

    if FLAGS["feat_bf16"]:
        features_dev = features.astype(ml_dtypes.bfloat16)
    else:
        features_dev = features

    in_maps = []
    for c in range(NC):
        m = dict(common)
        m["features"] = np.ascontiguousarray(
            features_dev[:, c * BPC:(c + 1) * BPC, :])
        in_maps.append(m)
    return in_maps


def kernel(features, w_sp1, b_sp1, w_sp2, b_sp2, w_rs1, b_rs1, w_rs2, b_rs2,
           decay_local, decay_global, w_ff, b_ff, ln_ff_g, ln_ff_b,
           w_gate, b_gate, ln_g, ln_b, _trace=False):
    inputs = dict(features=features, w_sp1=w_sp1, b_sp1=b_sp1, w_sp2=w_sp2,
                  b_sp2=b_sp2, w_rs1=w_rs1, b_rs1=b_rs1, w_rs2=w_rs2,
                  b_rs2=b_rs2, decay_local=decay_local,
                  decay_global=decay_global, w_ff=w_ff, b_ff=b_ff,
                  ln_ff_g=ln_ff_g, ln_ff_b=ln_ff_b, w_gate=w_gate,
                  b_gate=b_gate, ln_g=ln_g, ln_b=ln_b)
    features = np.asarray(features, np.float32)
    ln_g_np = np.asarray(ln_g, np.float32)
    ln_b_np = np.asarray(ln_b, np.float32)
    apply_ln_affine = not (np.all(ln_g_np == 1.0) and np.all(ln_b_np == 0.0))
    nc = _get_nc(apply_ln_affine)
    in_maps = build_in_maps(inputs)

    res = run_bass_kernel_spmd(nc, in_maps, core_ids=list(range(NC)),
                               trace=_trace)
    output = np.concatenate([np.asarray(res.results[c]["out"], np.float32)
                             for c in range(NC)], axis=1)
    attention_weights = np.full((S, B), 1.0 / S, dtype=features.dtype)
    if _trace:
        kernel.last_exec_time_ns = res.exec_time_ns
        kernel.last_trace = res.instructions_and_trace
    return output, attention_weights
